# revision 54
# baseline (speedup 1.0000x reference)
"""Trainium2 Bass kernel for GQA attention (dense_transformer).

Full module: x[1,2048,4096] -> causal GQA attention (32 q heads, 8 kv heads,
head_dim 128, RoPE) -> out[1,2048,4096].

Sharding: tensor-parallel by heads across 8 NeuronCores. Core c owns q heads
4c..4c+3 and kv head c; wq/wk/wv column-sharded, wo row-sharded; x replicated.
The trailing all-reduce over wo partial sums is done host-side (outputs are
gathered to host anyway).

v2: all matmul operands in bf16. fp32r streams 4-byte elements at
half rate through the PE (measured 390-430ns per 512-free matmul vs bf16's
~215ns); with tol 2e-2 and measured fp32r error 1.9e-4, bf16's ~1e-3 is safe.
Also: phase-1 weight DMA interleaved with the first chunk's matmuls (v1
stalled 61us preloading all weights), wo preloaded into SBUF during phase 2
(v1 phase 3 had 13 x ~10.6us PE gaps waiting on wo DMA + HAM cold restarts),
and softmax reciprocal via the ~5x faster reciprocal_approx_fast.

v4 (this file, 422us -> 390.6us): (1) softmax denominator moved off the PE
(was 160 of phase 2's 480 matmuls): the DVE accumulates bf16 sum(exp) per
pair (2-byte aps -> DVE 2X path; the Pool engine measured ~2us per
[128,1024] tensor op, 3x the DVE, and regressed to 542us) and one
ones-matmul per (h,j) broadcasts the cross-partition total. (2) score/PV
matmuls skip the causally dead sq prefix of diagonal sk-tiles (~22k PE
cycles/iter). (3) phase 3 fused into phase 2: chunk j-1's wo matmuls run
inside chunk j's ACT-gated attention window (8 d-tiles per iteration), so
the exp stream hides under wo work instead of stalling the PE; chunk 3
drains after the loop into per-d-tile 128KB DMAs (the final-transfer
latency is ~1us SP issue + ~5.8us of one-per-partition descriptors).
(4) fp16 output partials (range ~1e0, fp16 mantissa error ~1e-4 relative;
halves output DMA bytes); the host all-reduce upcasts to fp64. Measured
rel err 3.59e-3 vs the 2e-2 gate.

v4.1: each (h,j) iteration's 8-d-tile wo slice is emitted in two 4-d-tile
halves (iteration start + after the diagonal pairs) -- measured neutral
(391.8us band center) but spreads PE cover more evenly around the short
restricted diagonal PVs. NOTE for future measurement: a thermally
throttled device (after many back-to-back runs) reads ~467us for this
same binary with 273ns avg 512-matmuls vs 227ns; ~2-3 min idle restores
~391us.

fp8 (DoubleRow, 2x PE rate) was evaluated and is numerically dead here:
quantizing ANY single matmul operand to e4m3 alone produces 1.4-3.0e-2 on
the max-abs-err/max-abs metric (errors compose in quadrature; all-fp8 was
5.9e-2), and residual-corrected fp8 needs 2 matmuls = bf16 cost. Failed
experiments (all reverted): esum on the Pool engine (+120us), et split into
per-tile halves (+11us: 80 extra ACT ops x ~270ns overhead un-hide the exp
stream), partition-split final DMAs (+2us: extra SP issues cost more than
descriptor parallelism), 18 PE pre-warm filler matmuls (+1.2us).

On-chip layout notes:
  - All DRAM-side operands are pre-transposed on host so every matmul operand
    has its contraction dim on SBUF partitions with contiguous DMA patterns.
  - RoPE pairs are de-interleaved host-side (even rows then odd rows of each
    head of wq/wk), which turns the rotation into 64-partition-shifted
    multiply/adds on chip. Dot products are invariant to the permutation.
  - Scores are computed transposed (sk on partitions, sq on free) so the P@V
    matmul needs no on-chip transpose of the probabilities. The softmax
    denominator accumulates on the PE via an all-ones lhsT (broadcasts the
    column sum to every partition). Softmax max-subtraction is skipped:
    scores are O(+-10) here, exp cannot overflow in fp32.
"""

import math
from contextlib import ExitStack

import numpy as np
import ml_dtypes

import concourse.bass as bass
import concourse.mybir as mybir
import concourse.tile as tile
from concourse import bacc, bass_isa, bass_utils

F32 = mybir.dt.float32
BF16 = mybir.dt.bfloat16

# Full-scale config (hardcoded; kernel.py must be self-contained).
DIM = 4096
SEQ = 2048
N_HEADS = 32
N_KV_HEADS = 8
HEAD_DIM = 128
N_CORES = 8
HQ = N_HEADS // N_CORES            # q heads per core = 4
CH = 512                           # sq chunk (free dim of most matmuls)
SCALE = 1.0 / math.sqrt(HEAD_DIM)


def build_module(S=SEQ, D=DIM, hq=HQ, ch=CH):
    """Build the SPMD Bass/Tile module for one core's shard."""
    HD = HEAD_DIM
    H2 = HD // 2
    M = hq * HD                     # local q output dim
    R = ch // 128                   # sk-tiles per sq chunk
    nJ = S // ch                    # sq chunks
    nT = S // 128                   # sk tiles
    nD = D // 128                   # contraction tiles

    nc = bacc.Bacc("TRN2", target_bir_lowering=False, debug=False)
    xT = nc.dram_tensor("xT", [D, S], BF16, kind="ExternalInput").ap()
    wqkvT = nc.dram_tensor("wqkvT", [D, M + 2 * HD], BF16,
                           kind="ExternalInput").ap()
    woT = nc.dram_tensor("woT", [M, D], BF16, kind="ExternalInput").ap()
    constD = nc.dram_tensor("constD", [128, 256], BF16,
                            kind="ExternalInput").ap()
    cosP = nc.dram_tensor("cosP", [HD, S], F32, kind="ExternalInput").ap()
    sinP = nc.dram_tensor("sinP", [HD, S], F32, kind="ExternalInput").ap()
    maskD = nc.dram_tensor("maskD", [128, R * ch], BF16,
                           kind="ExternalInput").ap()
    outT = nc.dram_tensor("outT", [D, S], mybir.dt.float16,
                          kind="ExternalOutput").ap()

    with tile.TileContext(nc) as tc, ExitStack() as ctx, \
            nc.allow_low_precision(reason="bf16 staging for PE matmuls"):
        Exp = mybir.ActivationFunctionType.Exp

        # Per-CHUNK tiles for everything phase 2/3 read: Tile tracks RAW
        # deps at whole-tile granularity against the last emitted writer,
        # so a single [HD, S] kT made phase 2's first score matmul wait on
        # the LAST chunk's rope (~10us PE gap at the phase boundary).
        pers = ctx.enter_context(tc.tile_pool(name="pers", bufs=1))
        qc = [[pers.tile([HD, ch], BF16, tag=f"qc{h}_{j}", name=f"qc{h}_{j}")
               for j in range(nJ)] for h in range(hq)]
        kc = [pers.tile([HD, ch], BF16, tag=f"kc{j}", name=f"kc{j}")
              for j in range(nJ)]
        vc = [pers.tile([128, R * HD], BF16, tag=f"vc{j}", name=f"vc{j}")
              for j in range(nJ)]
        ident = pers.tile([128, 128], BF16, tag="ident", name="ident")
        ones128 = pers.tile([128, 128], BF16, tag="ones128", name="ones128")

        # Warm the ACT exp table at t=0: it otherwise lazy-loads (1.3us)
        # right in front of phase 2's first exp, on the critical hand-off.
        warm = pers.tile([128, 2], F32, tag="warm", name="warm")
        nc.gpsimd.memset(warm[:], 0.0)
        nc.scalar.activation(warm[:, 1:2], warm[:, 0:1],
                             mybir.ActivationFunctionType.Exp)

        rpool = ctx.enter_context(tc.tile_pool(name="rpool", bufs=2))

        def rope(out, ps, cj, sj):
            """out[:,chunk] = bf16(RoPE(ps)) with de-interleaved halves.

            The 64-partition swap always pairs a PSUM operand with an SBUF
            operand (mixed-space ops may differ in base partition; SB+SB
            ops must not — verifier checkSBSameStartPartition). Arithmetic
            in f32; only the final add writes bf16."""
            t1 = rpool.tile([HD, ch], F32, tag="ropet1", name="ropet1")
            t2 = rpool.tile([HD, ch], F32, tag="ropet2", name="ropet2")
            nc.vector.tensor_mul(t1[:], ps[:], cj)
            nc.vector.tensor_mul(t2[0:H2, :], ps[H2:HD, :], sj[0:H2, :])
            nc.vector.tensor_mul(t2[H2:HD, :], ps[0:H2, :], sj[H2:HD, :])
            nc.vector.tensor_add(out, t1[:], t2[:])

        # ---- Phase 1: QKV projections (+RoPE, +v transpose) ----
        # Weight DMA is interleaved with the j=0 chunk's matmuls so the PE
        # starts as soon as the first d-slice lands instead of after the
        # full preload. RoPE + v-transpose for chunk j are emitted AFTER
        # chunk j+1's matmul d-loop (program order = engine-queue order),
        # so the PE never parks behind the DVE at chunk boundaries; only
        # the 6 staging copies sit between chunks on the DVE.
        MW = M + 2 * HD
        wqkv_r = wqkvT.rearrange("(d p) m -> p d m", p=128)
        with tc.tile_pool(name="wpool", bufs=1) as wpool, \
             tc.tile_pool(name="xpool", bufs=2) as xpool, \
             tc.tile_pool(name="vpool", bufs=2) as vpool, \
             tc.tile_pool(name="qkv_ps", bufs=1, space="PSUM") as qkv_ps, \
             tc.tile_pool(name="vt_ps", bufs=2, space="PSUM") as vt_ps:
            wsb = wpool.tile([128, nD, MW], BF16, tag="wsb", name="wsb")
            cosb = wpool.tile([HD, S], F32, tag="cosb", name="cosb")
            sinb = wpool.tile([HD, S], F32, tag="sinb", name="sinb")

            # Each chunk runs TWO d-loop passes over SBUF-resident x:
            # pass A projects q heads (3 PSUM banks), pass B projects the
            # rest (3 banks). Pass A's rope chain drains on the DVE while
            # pass B's matmuls stream, so PSUM banks recycle without
            # parking the PE — with a single 6-bank pass, every chunk
            # boundary (and the phase-1 -> 2 hand-off) stalled ~2.3-12us
            # behind the rope chain's PSUM reads. The last chunk puts all
            # 4 q heads in pass A so the final pre-attention chain is just
            # rope(k) + the v transpose.
            xT_r = xT.rearrange("(a p) s -> p a s", p=128)

            # Phase-2 pair worklist + score emitter, defined here so the
            # first LOOK-ahead pairs can be EMITTED inside phase 1 right
            # after the last pass-B matmul: scheduled there, their PSUM
            # slots bind to the already-freed pass-A banks (the allocator
            # reuses the most-recently-freed banks, which otherwise makes
            # the first scores wait ~4us on the v-transpose chain) and the
            # PE stream stays dense enough across the hand-off that HAM
            # never drops to half clock.
            # Within each (h,j) iteration, the DIAGONAL pairs go first:
            # their et is ready only at exp(+1.1us)+mask(+0.7us DVE), and
            # with them last that latency stalled the PE ~0.5-0.7us twice
            # per iteration. First-in-order, their chains drain while the
            # PE consumes the non-diagonal pairs (exp-only, shorter chain).
            work = []          # (h, j, p, nP, first, last)
            for j2 in range(nJ):
                for h2 in range(hq):
                    nP2 = (j2 + 1) * R // 2
                    diag = [p2 for p2 in range(nP2)
                            if 2 * p2 - j2 * R >= 0]
                    nond = [p2 for p2 in range(nP2)
                            if 2 * p2 - j2 * R < 0]
                    seq = diag + nond
                    for k2, p2 in enumerate(seq):
                        work.append((h2, j2, p2, nP2,
                                     k2 == 0, k2 == nP2 - 1, k2 == 1))
            LOOK = 1
            pipe = []

            def emit_score(idx):
                """Score matmuls for pair idx, restricted to the causally
                live sq range of each sk tile (tile t only attends sq >=
                128*(t-R*j); start=True still zeroes the whole 2KB PSUM
                bank, so the skipped [0:lo) region reads as 0, not stale)."""
                h, j, p, nP = work[idx][:4]
                s_ps = attn_ps.tile([128, 2 * ch], F32, tag="sps",
                                    name="sps", bufs=LOOK + 1)
                for u in range(2):
                    t = 2 * p + u
                    lo = max(0, 128 * (t - R * j))
                    nc.tensor.matmul(
                        s_ps[:, u * ch + lo:(u + 1) * ch],
                        kc[t // R][:, (t % R) * 128:(t % R + 1) * 128],
                        qc[h][j][:, lo:],
                        start=True, stop=True)
                return s_ps

            def proj_pass(j, specs, xc, xdma, wdma):
                """One accumulation pass over d for `specs` =
                [(psum_tile, weight col offset), ...]. Weight DMA (chunk 0
                pass A only) is batched 4 d-slices per call: each
                dma_start costs ~650ns of Sync-queue issue time, and 32
                single-slice calls + x + consts made chunk-0's startup
                issue-bound (~6us PE stall)."""
                for d4 in range(nD // 4):
                    if d4 == 0 and wdma:
                        # kernel-start: interleave the first matmuls
                        # BETWEEN the head DMA issues. Tile's whole-tile
                        # RAW tracks only the last-emitted writer, so a
                        # matmul emitted after all three groups waits for
                        # ALL of them; emitted right after its own slice's
                        # dma_start it waits only that ~128-descriptor
                        # transfer (~3us sooner).
                        for dh, (a, b) in enumerate([(0, 1), (1, 2),
                                                     (2, 4)]):
                            nc.sync.dma_start(
                                xc[:, a:b, :], xT_r[:, a:b, 0:ch])
                            nc.sync.dma_start(
                                wsb[:, a:b, :], wqkv_r[:, a:b, :])
                            for d in range(a, b):
                                for ps, off in specs:
                                    nc.tensor.matmul(
                                        ps[:], wsb[:, d, off:off + HD],
                                        xc[:, d, :],
                                        start=(d == 0), stop=False)
                        # small consts early (needed by vtrans)
                        nc.sync.dma_start(ident[:], constD[:, 0:128])
                        nc.sync.dma_start(ones128[:], constD[:, 128:256])
                        continue
                    if xdma:
                        nc.sync.dma_start(
                            xc[:, 4 * d4:4 * d4 + 4, :],
                            xT_r[:, 4 * d4:4 * d4 + 4,
                                 j * ch:(j + 1) * ch])
                    if wdma:
                        nc.sync.dma_start(
                            wsb[:, 4 * d4:4 * d4 + 4, :],
                            wqkv_r[:, 4 * d4:4 * d4 + 4, :])
                    for dl in range(4):
                        d = 4 * d4 + dl
                        st, sp = (d == 0), (d == nD - 1)
                        xr = xc[:, d, :]
                        for ps, off in specs:
                            nc.tensor.matmul(
                                ps[:], wsb[:, d, off:off + HD], xr,
                                start=st, stop=sp)
                        if wdma and d == 24:
                            # cos/sin (2MB) late: first needed by the
                            # chunk-0 rope after pass A; issuing them
                            # early steals startup-critical bandwidth
                            nc.sync.dma_start(cosb[:], cosP[:])
                            nc.sync.dma_start(sinb[:], sinP[:])

            for j in range(nJ):
                cj = cosb[:, j * ch:(j + 1) * ch]
                sjs = sinb[:, j * ch:(j + 1) * ch]
                nqa = hq if j == nJ - 1 else 3   # q heads in pass A
                xc = xpool.tile([128, nD, ch], BF16, tag="xc", name="xc")
                # pass A: q heads (the last chunk folds q3 in; its 4th
                # bank borrows the pvt tag's budget, idle mid-chunk)
                ps_qa = [qkv_ps.tile([HD, ch], F32,
                                     tag=("psqb" if m == 3 else f"psqa{m}"),
                                     name=f"psqa{m}") for m in range(nqa)]
                proj_pass(j, [(ps_qa[m], m * HD) for m in range(nqa)],
                          xc, xdma=True, wdma=(j == 0))
                for m in range(nqa):
                    rope(qc[m][j][:], ps_qa[m], cj, sjs)
                # pass B: remaining q, k, v
                specs = []
                if nqa < hq:
                    ps_qb = qkv_ps.tile([HD, ch], F32, tag="psqb",
                                        name="psqb")
                    specs.append((ps_qb, 3 * HD))
                ps_k = qkv_ps.tile([HD, ch], F32, tag="psk", name="psk")
                ps_v = qkv_ps.tile([HD, ch], F32, tag="psv", name="psv")
                specs += [(ps_k, M), (ps_v, M + HD)]
                proj_pass(j, specs, xc, xdma=False, wdma=False)
                if nqa < hq:
                    rope(qc[3][j][:], ps_qb, cj, sjs)

                def vtrans(pad=False):
                    vt_s = vpool.tile([HD, ch], BF16, tag="vts", name="vts")
                    nc.vector.tensor_copy(vt_s[:], ps_v[:])
                    for r in range(R):
                        pvt = vt_ps.tile([128, 128], BF16, tag="pvt",
                                         name="pvt")
                        nc.tensor.transpose(
                            pvt[:], vt_s[:, r * 128:(r + 1) * 128],
                            ident[:])
                        nc.vector.tensor_copy(
                            vc[j][:, r * HD:(r + 1) * HD], pvt[:])
                        if pad:
                            # dummy transposes (outputs never read): dense
                            # PE activity across the phase hand-off so HAM
                            # doesn't re-throttle to half clock while the
                            # last rope's PSUM reads drain on the DVE
                            for _ in range(2):
                                nc.tensor.transpose(
                                    pvt[:], vt_s[:, r * 128:(r + 1) * 128],
                                    ident[:])

                if j < nJ - 1:
                    rope(kc[j][:], ps_k, cj, sjs)
                    vtrans()
                else:
                    # Last chunk: k-rope's PSUM-freeing muls first (psk's
                    # bank is reused by attention), then the v-transpose
                    # (frees psv + keeps the PE warm), and only the
                    # kc[3]-writing add last — kc[3] isn't read until
                    # attention's final column.
                    t1 = rpool.tile([HD, ch], F32, tag="ropet1",
                                    name="ropet1")
                    t2 = rpool.tile([HD, ch], F32, tag="ropet2",
                                    name="ropet2")
                    nc.vector.tensor_mul(t1[:], ps_k[:], cj)
                    nc.vector.tensor_mul(t2[0:H2, :], ps_k[H2:HD, :],
                                         sjs[0:H2, :])
                    nc.vector.tensor_mul(t2[H2:HD, :], ps_k[0:H2, :],
                                         sjs[H2:HD, :])
                    vtrans(pad=True)
                    nc.vector.tensor_add(kc[j][:], t1[:], t2[:])

        # ---- Phases 2+3 share the yT/mask/wo pool ----
        ypool = ctx.enter_context(tc.tile_pool(name="ypool", bufs=1))
        yc = [[ypool.tile([HD, ch], BF16, tag=f"yc{h}_{j}",
                          name=f"yc{h}_{j}") for j in range(nJ)]
              for h in range(hq)]
        maskb = ypool.tile([128, R * ch], BF16, tag="maskb", name="maskb")
        nc.sync.dma_start(maskb[:], maskD[:])
        # wo prefetch: 4MB bf16, lands in the first ~15us of phase 2's
        # ~100us of DMA-free compute; phase 3 then never waits on DMA.
        wo_sb = ypool.tile([128, hq, D], BF16, tag="wo_sb", name="wo_sb")
        for o in range(hq):
            nc.sync.dma_start(wo_sb[:, o, :], woT[o * 128:(o + 1) * 128, :])

        # ---- Phase 2: attention (transposed flash-style, causal) ----
        # sk-tiles are processed in PAIRS: both score matmuls land in one
        # [128, 2*ch] PSUM tile (2 banks) and a single ACTIVATE exps the
        # pair — the per-op ACT overhead (~260ns of the ~690ns a 512-tile
        # costs) was making phase 2 ACT-throughput-bound. nTj is always
        # even (R=4), and diagonal pairs align with mask pairs.
        apool = ctx.enter_context(tc.tile_pool(name="apool", bufs=6))
        npool = ctx.enter_context(tc.tile_pool(name="npool", bufs=2))
        opool = ctx.enter_context(tc.tile_pool(name="opool", bufs=2))
        outT_r = outT.rearrange("(a p) s -> p a s", p=128)
        with tc.tile_pool(name="attn_ps", bufs=2, space="PSUM") as attn_ps:
            # Single software pipeline ACROSS (h,j) iterations (worklist +
            # emit_score defined in phase 1; the first pairs were already
            # emitted there): without this, every iteration start stalled
            # the PE ~1.1us behind the exp of its first pair.
            #
            # v4: (1) the softmax denominator no longer runs on the PE (the
            # per-pair all-ones matmuls were 160 of phase 2's 480 matmuls):
            # the DVE accumulates sum(exp) per pair into a bf16 esum (all
            # aps 2-byte -> DVE 2X path), and one ones-matmul per (h,j) on
            # the folded sum broadcasts the cross-partition total. (2) score
            # and PV matmuls skip the causally dead sq prefix of diagonal
            # sk-tiles (start=True zeroes the whole 2KB PSUM region, so
            # dead score columns exp to 1 and the mask zeroes them; PV
            # stays full-width for j==0 where every pair is diagonal and a
            # restricted last matmul would leave y_ps accumulation regions
            # without a stop). (3) phase 3 is fused in: chunk j-1's wo
            # matmuls (32 d-tiles, 8 per (h,j) iteration) interleave into
            # chunk j's ACT-gated attention stream, and chunk 3's drain
            # after the loop. Output partials are fp16 (half the DMA bytes;
            # the host all-reduce upcasts), one DMA per chunk except the
            # last, which goes per-d-tile so the final transfer is 128KB.
            def emit_wo(j, dts):
                """wo matmuls+copy+DMA for chunk j over an 8-d-tile slice.
                Each slice gets its own fp16 staging tile and one 1MB
                strided DMA (a single per-chunk DMA landed in the drain
                window and added ~6us of tail); the last chunk goes
                per-d-tile so the final transfer is 128KB."""
                ot = opool.tile([128, len(dts), ch], mybir.dt.float16,
                                tag=f"osb{len(dts)}", name=f"osb{j}_{dts[0]}")
                for i, dt in enumerate(dts):
                    ps_o = attn_ps.tile([128, ch], F32, tag="pso",
                                        name="pso")
                    for o in range(hq):
                        nc.tensor.matmul(
                            ps_o[:],
                            wo_sb[:, o, dt * 128:(dt + 1) * 128],
                            yc[o][j][:],
                            start=(o == 0), stop=(o == hq - 1))
                    nc.vector.tensor_copy(ot[:, i, :], ps_o[:])
                    if j == nJ - 1:
                        nc.sync.dma_start(
                            outT_r[:, dt, j * ch:(j + 1) * ch],
                            ot[:, i, :])
                if j < nJ - 1:
                    nc.sync.dma_start(
                        outT_r[:, dts[0]:dts[-1] + 1,
                               j * ch:(j + 1) * ch],
                        ot[:])

            cur = None  # (y_ps, esum) for the open (h,j) iteration
            for idx, (h, j, p, nP, first, last, second) in enumerate(work):
                # Fill the score pipe BEFORE allocating yps: the first
                # tag to allocate gets the lowest PSUM banks, and the
                # attention pool's banks inherit hand-off deps from the
                # phase-1 tiles that owned them. sps-first puts the score
                # slots on the pass-A projection banks (freed mid-pass-B,
                # long before the boundary) instead of on psk/psv (freed
                # by the very last DVE ops), which cost ~4us + a HAM cold
                # restart at the phase hand-off.
                while len(pipe) <= idx and len(pipe) < len(work):
                    pipe.append(emit_score(len(pipe)))
                if first and j > 0:
                    # half of chunk j-1's wo slice; the other half lands
                    # after this iteration's diagonal pairs (below), where
                    # the PE otherwise outruns the ACT by ~0.5us (the diag
                    # pairs' restricted PVs are short) and idled waiting
                    # for the first non-diagonal et
                    emit_wo(j - 1, range(8 * h, 8 * h + 4))
                if first:
                    cur = (attn_ps.tile([HD, ch], F32, tag="yps",
                                        name="yps"),
                           npool.tile([128, 2 * ch], BF16, tag="esum",
                                      name="esum"))
                y_ps, esum = cur
                s_ps = pipe[idx]
                et = apool.tile([128, 2 * ch], BF16, tag="exp", name="et")
                # scale folded into wq host-side; ACT does pure exp
                nc.scalar.activation(et[:], s_ps[:], Exp)
                r0 = 2 * p - j * R
                if r0 >= 0:  # diagonal pair: apply causal mask
                    nc.vector.tensor_mul(
                        et[:], et[:], maskb[:, r0 * ch:(r0 + 2) * ch])
                while len(pipe) <= idx + LOOK and len(pipe) < len(work):
                    pipe.append(emit_score(len(pipe)))
                # denominator partial sums accumulate on the DVE
                # (post-mask et is zero in all dead/masked columns)
                if first:
                    nc.vector.tensor_copy(esum[:], et[:])
                else:
                    nc.vector.tensor_add(esum[:], esum[:], et[:])
                for u in range(2):
                    t = 2 * p + u
                    lo = max(0, 128 * (t - R * j)) if j > 0 else 0
                    st2, sp2 = (first and u == 0), (last and u == 1)
                    nc.tensor.matmul(
                        y_ps[:, lo:],
                        vc[t // R][:, (t % R) * HD:(t % R + 1) * HD],
                        et[:, u * ch + lo:(u + 1) * ch],
                        start=st2, stop=sp2)
                if second and j > 0:
                    emit_wo(j - 1, range(8 * h + 4, 8 * h + 8))
                if last:
                    esf = npool.tile([128, ch], BF16, tag="esf",
                                     name="esf")
                    nc.vector.tensor_add(esf[:], esum[:, 0:ch],
                                         esum[:, ch:])
                    ps_d = attn_ps.tile([128, ch], F32, tag="pso",
                                        name="dps")
                    nc.tensor.matmul(ps_d[:], ones128[:], esf[:],
                                     start=True, stop=True)
                    rec = npool.tile([128, ch], F32, tag="rec", name="rec")
                    nc.vector.reciprocal_approx_fast(rec[:], ps_d[:])
                    nc.vector.tensor_mul(yc[h][j][:], y_ps[:], rec[:])
            # drain: last chunk's output projection (pure PE+DMA streaming,
            # no ACT left -- also replaces the old phase-3 warmth fillers)
            for sl in range(nD // 8):
                emit_wo(nJ - 1, range(8 * sl, 8 * sl + 8))
    nc.compile()
    return nc


def _deinterleave_perm(hd):
    """Row permutation putting even indices first, odd second."""
    return np.concatenate([np.arange(0, hd, 2), np.arange(1, hd, 2)])


def host_prep(x, wq, wk, wv, wo, freqs_cos, freqs_sin,
              n_cores=N_CORES, hq=HQ, n_kv=N_KV_HEADS):
    """Build the per-core input maps (numpy, host-side)."""
    HD = HEAD_DIM
    D = x.shape[-1]
    S = x.shape[-2]
    M = hq * HD
    R = CH // 128
    BF = ml_dtypes.bfloat16
    x = np.asarray(x, np.float32).reshape(S, D)
    wq = np.asarray(wq, np.float32)
    wk = np.asarray(wk, np.float32)
    wv = np.asarray(wv, np.float32)
    wo = np.asarray(wo, np.float32)
    fc = np.asarray(freqs_cos, np.float32)
    fs = np.asarray(freqs_sin, np.float32)

    perm = _deinterleave_perm(HD)
    wq = wq * np.float32(SCALE)   # fold softmax scale into q projection
    xT = np.ascontiguousarray(x.T.astype(BF))           # [D, S] bf16
    cosP = np.ascontiguousarray(np.concatenate([fc.T, fc.T], 0))  # [128, S]
    sinP = np.ascontiguousarray(np.concatenate([-fs.T, fs.T], 0))
    # mask[t, r*CH + s] = 1 if 128*r + t <= s else 0
    tt = np.arange(128)[:, None]
    ss = np.arange(CH)[None, :]
    maskD = np.concatenate(
        [(128 * r + tt <= ss).astype(np.float32) for r in range(R)], axis=1)
    maskD = np.ascontiguousarray(maskD.astype(BF))      # [128, R*CH] bf16
    constD = np.concatenate(
        [np.eye(128, dtype=np.float32), np.ones((128, 128), np.float32)],
        axis=1).astype(BF)                              # [128, 256] bf16

    in_maps = []
    for c in range(n_cores):
        wq_c = wq[c * M:(c + 1) * M, :].reshape(hq, HD, D)[:, perm, :]
        wq_c = wq_c.reshape(M, D)
        wk_c = wk[c * HD:(c + 1) * HD, :][perm, :]
        wv_c = wv[c * HD:(c + 1) * HD, :]
        wqkvT = np.ascontiguousarray(
            np.concatenate([wq_c, wk_c, wv_c], axis=0).T.astype(BF))
        woT = np.ascontiguousarray(
            wo[:, c * M:(c + 1) * M].T.astype(BF))      # [M, D] bf16
        in_maps.append({
            "xT": xT, "wqkvT": wqkvT, "woT": woT, "constD": constD,
            "cosP": cosP, "sinP": sinP, "maskD": maskD,
        })
    return in_maps


_NC_CACHE = {}


def _get_module():
    if "nc" not in _NC_CACHE:
        _NC_CACHE["nc"] = build_module()
    return _NC_CACHE["nc"]


def run_on_cores(in_maps, trace=False):
    nc = _get_module()
    res = bass_utils.run_bass_kernel_spmd(
        nc, in_maps, core_ids=list(range(len(in_maps))), trace=trace)
    return res


def kernel(x, wq, wk, wv, wo, freqs_cos, freqs_sin):
    in_maps = host_prep(x, wq, wk, wv, wo, freqs_cos, freqs_sin)
    res = run_on_cores(in_maps)
    acc = None
    for r in res.results:
        o = r["outT"]
        acc = o.astype(np.float64) if acc is None else acc + o
    out = acc.T.astype(np.float32).reshape(1, SEQ, DIM)
    return out



# revision 65
# speedup vs baseline: 1.0077x; 1.0077x over previous
"""Trainium2 Bass kernel for GQA attention (dense_transformer).

Full module: x[1,2048,4096] -> causal GQA attention (32 q heads, 8 kv heads,
head_dim 128, RoPE) -> out[1,2048,4096].

Sharding: tensor-parallel by heads across 8 NeuronCores. Core c owns q heads
4c..4c+3 and kv head c; wq/wk/wv column-sharded, wo row-sharded; x replicated.
The trailing all-reduce over wo partial sums is done host-side (outputs are
gathered to host anyway).

v2: all matmul operands in bf16. fp32r streams 4-byte elements at
half rate through the PE (measured 390-430ns per 512-free matmul vs bf16's
~215ns); with tol 2e-2 and measured fp32r error 1.9e-4, bf16's ~1e-3 is safe.
Also: phase-1 weight DMA interleaved with the first chunk's matmuls (v1
stalled 61us preloading all weights), wo preloaded into SBUF during phase 2
(v1 phase 3 had 13 x ~10.6us PE gaps waiting on wo DMA + HAM cold restarts),
and softmax reciprocal via the ~5x faster reciprocal_approx_fast.

v4 (this file, 422us -> 390.6us): (1) softmax denominator moved off the PE
(was 160 of phase 2's 480 matmuls): the DVE accumulates bf16 sum(exp) per
pair (2-byte aps -> DVE 2X path; the Pool engine measured ~2us per
[128,1024] tensor op, 3x the DVE, and regressed to 542us) and one
ones-matmul per (h,j) broadcasts the cross-partition total. (2) score/PV
matmuls skip the causally dead sq prefix of diagonal sk-tiles (~22k PE
cycles/iter). (3) phase 3 fused into phase 2: chunk j-1's wo matmuls run
inside chunk j's ACT-gated attention window (8 d-tiles per iteration), so
the exp stream hides under wo work instead of stalling the PE; chunk 3
drains after the loop into per-d-tile 128KB DMAs (the final-transfer
latency is ~1us SP issue + ~5.8us of one-per-partition descriptors).
(4) fp16 output partials (range ~1e0, fp16 mantissa error ~1e-4 relative;
halves output DMA bytes); the host all-reduce upcasts to fp64. Measured
rel err 3.59e-3 vs the 2e-2 gate.

v4.1: each (h,j) iteration's 8-d-tile wo slice is emitted in two 4-d-tile
halves (iteration start + after the diagonal pairs) -- measured neutral
(391.8us band center) but spreads PE cover more evenly around the short
restricted diagonal PVs.

v4.2 (390.46us best measurement): the kernel-start matmuls are emitted
BETWEEN the head DMA issues, not after them -- Tile's whole-tile RAW
tracking pins a read to the last-emitted writer, so a matmul emitted
after all three head groups waited for all of them; emitted right after
its own slice's dma_start it waits only that ~128-descriptor transfer.
Head grouping stays (0:1),(1:2),(2:4): splitting (2:4) into singles
measured +2.4us (extra SP issue slots delay later groups). NOTE for future measurement: a thermally
throttled device (after many back-to-back runs) reads ~467us for this
same binary with 273ns avg 512-matmuls vs 227ns; ~2-3 min idle restores
~391us.

fp8 (DoubleRow, 2x PE rate) was evaluated and is numerically dead here:
quantizing ANY single matmul operand to e4m3 alone produces 1.4-3.0e-2 on
the max-abs-err/max-abs metric (errors compose in quadrature; all-fp8 was
5.9e-2), and residual-corrected fp8 needs 2 matmuls = bf16 cost. Failed
experiments (all reverted): esum on the Pool engine (+120us), et split into
per-tile halves (+11us: 80 extra ACT ops x ~270ns overhead un-hide the exp
stream), partition-split final DMAs (+2us: extra SP issues cost more than
descriptor parallelism), 18 PE pre-warm filler matmuls (+1.2us).

On-chip layout notes:
  - All DRAM-side operands are pre-transposed on host so every matmul operand
    has its contraction dim on SBUF partitions with contiguous DMA patterns.
  - RoPE pairs are de-interleaved host-side (even rows then odd rows of each
    head of wq/wk), which turns the rotation into 64-partition-shifted
    multiply/adds on chip. Dot products are invariant to the permutation.
  - Scores are computed transposed (sk on partitions, sq on free) so the P@V
    matmul needs no on-chip transpose of the probabilities. The softmax
    denominator accumulates on the PE via an all-ones lhsT (broadcasts the
    column sum to every partition). Softmax max-subtraction is skipped:
    scores are O(+-10) here, exp cannot overflow in fp32.
"""

import math
from contextlib import ExitStack

import numpy as np
import ml_dtypes

import concourse.bass as bass
import concourse.mybir as mybir
import concourse.tile as tile
from concourse import bacc, bass_isa, bass_utils

F32 = mybir.dt.float32
BF16 = mybir.dt.bfloat16

# Full-scale config (hardcoded; kernel.py must be self-contained).
DIM = 4096
SEQ = 2048
N_HEADS = 32
N_KV_HEADS = 8
HEAD_DIM = 128
N_CORES = 8
HQ = N_HEADS // N_CORES            # q heads per core = 4
CH = 512                           # sq chunk (free dim of most matmuls)
SCALE = 1.0 / math.sqrt(HEAD_DIM)


def build_module(S=SEQ, D=DIM, hq=HQ, ch=CH):
    """Build the SPMD Bass/Tile module for one core's shard."""
    HD = HEAD_DIM
    H2 = HD // 2
    M = hq * HD                     # local q output dim
    R = ch // 128                   # sk-tiles per sq chunk
    nJ = S // ch                    # sq chunks
    nT = S // 128                   # sk tiles
    nD = D // 128                   # contraction tiles

    nc = bacc.Bacc("TRN2", target_bir_lowering=False, debug=False)
    xT = nc.dram_tensor("xT", [D, S], BF16, kind="ExternalInput").ap()
    wqkvT = nc.dram_tensor("wqkvT", [D, M + 2 * HD], BF16,
                           kind="ExternalInput").ap()
    woT = nc.dram_tensor("woT", [M, D], BF16, kind="ExternalInput").ap()
    constD = nc.dram_tensor("constD", [128, 256], BF16,
                            kind="ExternalInput").ap()
    cosP = nc.dram_tensor("cosP", [HD, S], F32, kind="ExternalInput").ap()
    sinP = nc.dram_tensor("sinP", [HD, S], F32, kind="ExternalInput").ap()
    maskD = nc.dram_tensor("maskD", [128, R * ch], BF16,
                           kind="ExternalInput").ap()
    outT = nc.dram_tensor("outT", [D, S], mybir.dt.float16,
                          kind="ExternalOutput").ap()

    with tile.TileContext(nc) as tc, ExitStack() as ctx, \
            nc.allow_low_precision(reason="bf16 staging for PE matmuls"):
        Exp = mybir.ActivationFunctionType.Exp

        # Per-CHUNK tiles for everything phase 2/3 read: Tile tracks RAW
        # deps at whole-tile granularity against the last emitted writer,
        # so a single [HD, S] kT made phase 2's first score matmul wait on
        # the LAST chunk's rope (~10us PE gap at the phase boundary).
        pers = ctx.enter_context(tc.tile_pool(name="pers", bufs=1))
        qc = [[pers.tile([HD, ch], BF16, tag=f"qc{h}_{j}", name=f"qc{h}_{j}")
               for j in range(nJ)] for h in range(hq)]
        kc = [pers.tile([HD, ch], BF16, tag=f"kc{j}", name=f"kc{j}")
              for j in range(nJ)]
        vc = [pers.tile([128, R * HD], BF16, tag=f"vc{j}", name=f"vc{j}")
              for j in range(nJ)]
        ident = pers.tile([128, 128], BF16, tag="ident", name="ident")
        ones128 = pers.tile([128, 128], BF16, tag="ones128", name="ones128")

        # Warm the ACT exp table at t=0: it otherwise lazy-loads (1.3us)
        # right in front of phase 2's first exp, on the critical hand-off.
        warm = pers.tile([128, 2], F32, tag="warm", name="warm")
        nc.gpsimd.memset(warm[:], 0.0)
        nc.scalar.activation(warm[:, 1:2], warm[:, 0:1],
                             mybir.ActivationFunctionType.Exp)

        rpool = ctx.enter_context(tc.tile_pool(name="rpool", bufs=2))

        def rope(out, ps, cj, sj):
            """out[:,chunk] = bf16(RoPE(ps)) with de-interleaved halves.

            The 64-partition swap always pairs a PSUM operand with an SBUF
            operand (mixed-space ops may differ in base partition; SB+SB
            ops must not — verifier checkSBSameStartPartition). Arithmetic
            in f32; only the final add writes bf16."""
            t1 = rpool.tile([HD, ch], F32, tag="ropet1", name="ropet1")
            t2 = rpool.tile([HD, ch], F32, tag="ropet2", name="ropet2")
            nc.vector.tensor_mul(t1[:], ps[:], cj)
            nc.vector.tensor_mul(t2[0:H2, :], ps[H2:HD, :], sj[0:H2, :])
            nc.vector.tensor_mul(t2[H2:HD, :], ps[0:H2, :], sj[H2:HD, :])
            nc.vector.tensor_add(out, t1[:], t2[:])

        # ---- Phase 1: QKV projections (+RoPE, +v transpose) ----
        # Weight DMA is interleaved with the j=0 chunk's matmuls so the PE
        # starts as soon as the first d-slice lands instead of after the
        # full preload. RoPE + v-transpose for chunk j are emitted AFTER
        # chunk j+1's matmul d-loop (program order = engine-queue order),
        # so the PE never parks behind the DVE at chunk boundaries; only
        # the 6 staging copies sit between chunks on the DVE.
        MW = M + 2 * HD
        wqkv_r = wqkvT.rearrange("(d p) m -> p d m", p=128)
        with tc.tile_pool(name="wpool", bufs=1) as wpool, \
             tc.tile_pool(name="xpool", bufs=2) as xpool, \
             tc.tile_pool(name="vpool", bufs=2) as vpool, \
             tc.tile_pool(name="qkv_ps", bufs=1, space="PSUM") as qkv_ps, \
             tc.tile_pool(name="vt_ps", bufs=2, space="PSUM") as vt_ps:
            wsb = wpool.tile([128, nD, MW], BF16, tag="wsb", name="wsb")
            cosb = wpool.tile([HD, S], F32, tag="cosb", name="cosb")
            sinb = wpool.tile([HD, S], F32, tag="sinb", name="sinb")

            # Each chunk runs TWO d-loop passes over SBUF-resident x:
            # pass A projects q heads (3 PSUM banks), pass B projects the
            # rest (3 banks). Pass A's rope chain drains on the DVE while
            # pass B's matmuls stream, so PSUM banks recycle without
            # parking the PE — with a single 6-bank pass, every chunk
            # boundary (and the phase-1 -> 2 hand-off) stalled ~2.3-12us
            # behind the rope chain's PSUM reads. The last chunk puts all
            # 4 q heads in pass A so the final pre-attention chain is just
            # rope(k) + the v transpose.
            xT_r = xT.rearrange("(a p) s -> p a s", p=128)

            # Phase-2 pair worklist + score emitter, defined here so the
            # first LOOK-ahead pairs can be EMITTED inside phase 1 right
            # after the last pass-B matmul: scheduled there, their PSUM
            # slots bind to the already-freed pass-A banks (the allocator
            # reuses the most-recently-freed banks, which otherwise makes
            # the first scores wait ~4us on the v-transpose chain) and the
            # PE stream stays dense enough across the hand-off that HAM
            # never drops to half clock.
            # Within each (h,j) iteration, the DIAGONAL pairs go first:
            # their et is ready only at exp(+1.1us)+mask(+0.7us DVE), and
            # with them last that latency stalled the PE ~0.5-0.7us twice
            # per iteration. First-in-order, their chains drain while the
            # PE consumes the non-diagonal pairs (exp-only, shorter chain).
            work = []          # (h, j, p, nP, first, last)
            for j2 in range(nJ):
                for h2 in range(hq):
                    nP2 = (j2 + 1) * R // 2
                    diag = [p2 for p2 in range(nP2)
                            if 2 * p2 - j2 * R >= 0]
                    nond = [p2 for p2 in range(nP2)
                            if 2 * p2 - j2 * R < 0]
                    seq = diag + nond
                    for k2, p2 in enumerate(seq):
                        work.append((h2, j2, p2, nP2,
                                     k2 == 0, k2 == nP2 - 1, k2 == 1))
            LOOK = 1
            pipe = []

            def emit_score(idx):
                """Score matmuls for pair idx, restricted to the causally
                live sq range of each sk tile (tile t only attends sq >=
                128*(t-R*j); start=True still zeroes the whole 2KB PSUM
                bank, so the skipped [0:lo) region reads as 0, not stale)."""
                h, j, p, nP = work[idx][:4]
                s_ps = attn_ps.tile([128, 2 * ch], F32, tag="sps",
                                    name="sps", bufs=LOOK + 1)
                for u in range(2):
                    t = 2 * p + u
                    lo = max(0, 128 * (t - R * j))
                    nc.tensor.matmul(
                        s_ps[:, u * ch + lo:(u + 1) * ch],
                        kc[t // R][:, (t % R) * 128:(t % R + 1) * 128],
                        qc[h][j][:, lo:],
                        start=True, stop=True)
                return s_ps

            def proj_pass(j, specs, xc, xdma, wdma):
                """One accumulation pass over d for `specs` =
                [(psum_tile, weight col offset), ...]. Weight DMA (chunk 0
                pass A only) is batched 4 d-slices per call: each
                dma_start costs ~650ns of Sync-queue issue time, and 32
                single-slice calls + x + consts made chunk-0's startup
                issue-bound (~6us PE stall)."""
                for d4 in range(nD // 4):
                    if d4 == 0 and wdma:
                        # kernel-start: interleave the first matmuls
                        # BETWEEN the head DMA issues. Tile's whole-tile
                        # RAW tracks only the last-emitted writer, so a
                        # matmul emitted after all three groups waits for
                        # ALL of them; emitted right after its own slice's
                        # dma_start it waits only that ~128-descriptor
                        # transfer (~3us sooner).
                        for dh, (a, b) in enumerate([(0, 1), (1, 2),
                                                     (2, 4)]):
                            nc.sync.dma_start(
                                xc[:, a:b, :], xT_r[:, a:b, 0:ch])
                            nc.sync.dma_start(
                                wsb[:, a:b, :], wqkv_r[:, a:b, :])
                            for d in range(a, b):
                                for ps, off in specs:
                                    nc.tensor.matmul(
                                        ps[:], wsb[:, d, off:off + HD],
                                        xc[:, d, :],
                                        start=(d == 0), stop=False)
                        # small consts early (needed by vtrans)
                        nc.sync.dma_start(ident[:], constD[:, 0:128])
                        nc.sync.dma_start(ones128[:], constD[:, 128:256])
                        continue
                    if xdma:
                        nc.sync.dma_start(
                            xc[:, 4 * d4:4 * d4 + 4, :],
                            xT_r[:, 4 * d4:4 * d4 + 4,
                                 j * ch:(j + 1) * ch])
                    if wdma:
                        nc.sync.dma_start(
                            wsb[:, 4 * d4:4 * d4 + 4, :],
                            wqkv_r[:, 4 * d4:4 * d4 + 4, :])
                    for dl in range(4):
                        d = 4 * d4 + dl
                        st, sp = (d == 0), (d == nD - 1)
                        xr = xc[:, d, :]
                        for ps, off in specs:
                            nc.tensor.matmul(
                                ps[:], wsb[:, d, off:off + HD], xr,
                                start=st, stop=sp)
                        if wdma and d == 24:
                            # cos/sin (2MB) late: first needed by the
                            # chunk-0 rope after pass A; issuing them
                            # early steals startup-critical bandwidth
                            nc.sync.dma_start(cosb[:], cosP[:])
                            nc.sync.dma_start(sinb[:], sinP[:])

            for j in range(nJ):
                cj = cosb[:, j * ch:(j + 1) * ch]
                sjs = sinb[:, j * ch:(j + 1) * ch]
                nqa = hq if j == nJ - 1 else 3   # q heads in pass A
                xc = xpool.tile([128, nD, ch], BF16, tag="xc", name="xc")
                # pass A: q heads (the last chunk folds q3 in; its 4th
                # bank borrows the pvt tag's budget, idle mid-chunk)
                ps_qa = [qkv_ps.tile([HD, ch], F32,
                                     tag=("psqb" if m == 3 else f"psqa{m}"),
                                     name=f"psqa{m}") for m in range(nqa)]
                proj_pass(j, [(ps_qa[m], m * HD) for m in range(nqa)],
                          xc, xdma=True, wdma=(j == 0))
                for m in range(nqa):
                    rope(qc[m][j][:], ps_qa[m], cj, sjs)
                # pass B: remaining q, k, v
                specs = []
                if nqa < hq:
                    ps_qb = qkv_ps.tile([HD, ch], F32, tag="psqb",
                                        name="psqb")
                    specs.append((ps_qb, 3 * HD))
                ps_k = qkv_ps.tile([HD, ch], F32, tag="psk", name="psk")
                ps_v = qkv_ps.tile([HD, ch], F32, tag="psv", name="psv")
                specs += [(ps_k, M), (ps_v, M + HD)]
                proj_pass(j, specs, xc, xdma=False, wdma=False)
                if nqa < hq:
                    rope(qc[3][j][:], ps_qb, cj, sjs)

                def vtrans(pad=False):
                    vt_s = vpool.tile([HD, ch], BF16, tag="vts", name="vts")
                    nc.vector.tensor_copy(vt_s[:], ps_v[:])
                    for r in range(R):
                        pvt = vt_ps.tile([128, 128], BF16, tag="pvt",
                                         name="pvt")
                        nc.tensor.transpose(
                            pvt[:], vt_s[:, r * 128:(r + 1) * 128],
                            ident[:])
                        nc.vector.tensor_copy(
                            vc[j][:, r * HD:(r + 1) * HD], pvt[:])
                        if pad:
                            # dummy transposes (outputs never read): dense
                            # PE activity across the phase hand-off so HAM
                            # doesn't re-throttle to half clock while the
                            # last rope's PSUM reads drain on the DVE
                            for _ in range(2):
                                nc.tensor.transpose(
                                    pvt[:], vt_s[:, r * 128:(r + 1) * 128],
                                    ident[:])

                if j < nJ - 1:
                    rope(kc[j][:], ps_k, cj, sjs)
                    vtrans()
                else:
                    # Last chunk: k-rope's PSUM-freeing muls first (psk's
                    # bank is reused by attention), then the v-transpose
                    # (frees psv + keeps the PE warm), and only the
                    # kc[3]-writing add last — kc[3] isn't read until
                    # attention's final column.
                    t1 = rpool.tile([HD, ch], F32, tag="ropet1",
                                    name="ropet1")
                    t2 = rpool.tile([HD, ch], F32, tag="ropet2",
                                    name="ropet2")
                    nc.vector.tensor_mul(t1[:], ps_k[:], cj)
                    nc.vector.tensor_mul(t2[0:H2, :], ps_k[H2:HD, :],
                                         sjs[0:H2, :])
                    nc.vector.tensor_mul(t2[H2:HD, :], ps_k[0:H2, :],
                                         sjs[H2:HD, :])
                    vtrans(pad=True)
                    nc.vector.tensor_add(kc[j][:], t1[:], t2[:])

        # ---- Phases 2+3 share the yT/mask/wo pool ----
        ypool = ctx.enter_context(tc.tile_pool(name="ypool", bufs=1))
        yc = [[ypool.tile([HD, ch], BF16, tag=f"yc{h}_{j}",
                          name=f"yc{h}_{j}") for j in range(nJ)]
              for h in range(hq)]
        maskb = ypool.tile([128, R * ch], BF16, tag="maskb", name="maskb")
        nc.sync.dma_start(maskb[:], maskD[:])
        # wo prefetch: 4MB bf16, lands in the first ~15us of phase 2's
        # ~100us of DMA-free compute; phase 3 then never waits on DMA.
        wo_sb = ypool.tile([128, hq, D], BF16, tag="wo_sb", name="wo_sb")
        for o in range(hq):
            nc.sync.dma_start(wo_sb[:, o, :], woT[o * 128:(o + 1) * 128, :])

        # ---- Phase 2: attention (transposed flash-style, causal) ----
        # sk-tiles are processed in PAIRS: both score matmuls land in one
        # [128, 2*ch] PSUM tile (2 banks) and a single ACTIVATE exps the
        # pair — the per-op ACT overhead (~260ns of the ~690ns a 512-tile
        # costs) was making phase 2 ACT-throughput-bound. nTj is always
        # even (R=4), and diagonal pairs align with mask pairs.
        apool = ctx.enter_context(tc.tile_pool(name="apool", bufs=6))
        npool = ctx.enter_context(tc.tile_pool(name="npool", bufs=2))
        opool = ctx.enter_context(tc.tile_pool(name="opool", bufs=2))
        outT_r = outT.rearrange("(a p) s -> p a s", p=128)
        with tc.tile_pool(name="attn_ps", bufs=2, space="PSUM") as attn_ps:
            # Single software pipeline ACROSS (h,j) iterations (worklist +
            # emit_score defined in phase 1; the first pairs were already
            # emitted there): without this, every iteration start stalled
            # the PE ~1.1us behind the exp of its first pair.
            #
            # v4: (1) the softmax denominator no longer runs on the PE (the
            # per-pair all-ones matmuls were 160 of phase 2's 480 matmuls):
            # the DVE accumulates sum(exp) per pair into a bf16 esum (all
            # aps 2-byte -> DVE 2X path), and one ones-matmul per (h,j) on
            # the folded sum broadcasts the cross-partition total. (2) score
            # and PV matmuls skip the causally dead sq prefix of diagonal
            # sk-tiles (start=True zeroes the whole 2KB PSUM region, so
            # dead score columns exp to 1 and the mask zeroes them; PV
            # stays full-width for j==0 where every pair is diagonal and a
            # restricted last matmul would leave y_ps accumulation regions
            # without a stop). (3) phase 3 is fused in: chunk j-1's wo
            # matmuls (32 d-tiles, 8 per (h,j) iteration) interleave into
            # chunk j's ACT-gated attention stream, and chunk 3's drain
            # after the loop. Output partials are fp16 (half the DMA bytes;
            # the host all-reduce upcasts), one DMA per chunk except the
            # last, which goes per-d-tile so the final transfer is 128KB.
            def emit_wo(j, dts):
                """wo matmuls+copy+DMA for chunk j over an 8-d-tile slice.
                Each slice gets its own fp16 staging tile and one 1MB
                strided DMA (a single per-chunk DMA landed in the drain
                window and added ~6us of tail); the last chunk goes
                per-d-tile so the final transfer is 128KB."""
                ot = opool.tile([128, len(dts), ch], mybir.dt.float16,
                                tag=f"osb{len(dts)}", name=f"osb{j}_{dts[0]}")
                for i, dt in enumerate(dts):
                    ps_o = attn_ps.tile([128, ch], F32, tag="pso",
                                        name="pso")
                    for o in range(hq):
                        nc.tensor.matmul(
                            ps_o[:],
                            wo_sb[:, o, dt * 128:(dt + 1) * 128],
                            yc[o][j][:],
                            start=(o == 0), stop=(o == hq - 1))
                    nc.vector.tensor_copy(ot[:, i, :], ps_o[:])
                    if j == nJ - 1:
                        nc.sync.dma_start(
                            outT_r[:, dt, j * ch:(j + 1) * ch],
                            ot[:, i, :])
                if j < nJ - 1:
                    nc.sync.dma_start(
                        outT_r[:, dts[0]:dts[-1] + 1,
                               j * ch:(j + 1) * ch],
                        ot[:])

            cur = None  # (y_ps, esum) for the open (h,j) iteration
            for idx, (h, j, p, nP, first, last, second) in enumerate(work):
                # Fill the score pipe BEFORE allocating yps: the first
                # tag to allocate gets the lowest PSUM banks, and the
                # attention pool's banks inherit hand-off deps from the
                # phase-1 tiles that owned them. sps-first puts the score
                # slots on the pass-A projection banks (freed mid-pass-B,
                # long before the boundary) instead of on psk/psv (freed
                # by the very last DVE ops), which cost ~4us + a HAM cold
                # restart at the phase hand-off.
                while len(pipe) <= idx and len(pipe) < len(work):
                    pipe.append(emit_score(len(pipe)))
                if first and j > 0:
                    # half of chunk j-1's wo slice; the other half lands
                    # after this iteration's diagonal pairs (below), where
                    # the PE otherwise outruns the ACT by ~0.5us (the diag
                    # pairs' restricted PVs are short) and idled waiting
                    # for the first non-diagonal et
                    emit_wo(j - 1, range(8 * h, 8 * h + 4))
                if first:
                    cur = (attn_ps.tile([HD, ch], F32, tag="yps",
                                        name="yps"),
                           npool.tile([128, 2 * ch], BF16, tag="esum",
                                      name="esum"))
                y_ps, esum = cur
                s_ps = pipe[idx]
                et = apool.tile([128, 2 * ch], BF16, tag="exp", name="et")
                # scale folded into wq host-side; ACT does pure exp
                nc.scalar.activation(et[:], s_ps[:], Exp)
                r0 = 2 * p - j * R
                if r0 >= 0:  # diagonal pair: apply causal mask
                    nc.vector.tensor_mul(
                        et[:], et[:], maskb[:, r0 * ch:(r0 + 2) * ch])
                while len(pipe) <= idx + LOOK and len(pipe) < len(work):
                    pipe.append(emit_score(len(pipe)))
                # denominator partial sums accumulate on the DVE
                # (post-mask et is zero in all dead/masked columns)
                if first:
                    nc.vector.tensor_copy(esum[:], et[:])
                else:
                    nc.vector.tensor_add(esum[:], esum[:], et[:])
                for u in range(2):
                    t = 2 * p + u
                    lo = max(0, 128 * (t - R * j)) if j > 0 else 0
                    st2, sp2 = (first and u == 0), (last and u == 1)
                    nc.tensor.matmul(
                        y_ps[:, lo:],
                        vc[t // R][:, (t % R) * HD:(t % R + 1) * HD],
                        et[:, u * ch + lo:(u + 1) * ch],
                        start=st2, stop=sp2)
                if second and j > 0:
                    emit_wo(j - 1, range(8 * h + 4, 8 * h + 8))
                if last:
                    esf = npool.tile([128, ch], BF16, tag="esf",
                                     name="esf")
                    nc.vector.tensor_add(esf[:], esum[:, 0:ch],
                                         esum[:, ch:])
                    ps_d = attn_ps.tile([128, ch], F32, tag="pso",
                                        name="dps")
                    nc.tensor.matmul(ps_d[:], ones128[:], esf[:],
                                     start=True, stop=True)
                    rec = npool.tile([128, ch], F32, tag="rec", name="rec")
                    nc.vector.reciprocal_approx_fast(rec[:], ps_d[:])
                    nc.vector.tensor_mul(yc[h][j][:], y_ps[:], rec[:])
            # drain: last chunk's output projection (pure PE+DMA streaming,
            # no ACT left -- also replaces the old phase-3 warmth fillers)
            for sl in range(nD // 8):
                emit_wo(nJ - 1, range(8 * sl, 8 * sl + 8))
    nc.compile()
    return nc


def _deinterleave_perm(hd):
    """Row permutation putting even indices first, odd second."""
    return np.concatenate([np.arange(0, hd, 2), np.arange(1, hd, 2)])


def host_prep(x, wq, wk, wv, wo, freqs_cos, freqs_sin,
              n_cores=N_CORES, hq=HQ, n_kv=N_KV_HEADS):
    """Build the per-core input maps (numpy, host-side)."""
    HD = HEAD_DIM
    D = x.shape[-1]
    S = x.shape[-2]
    M = hq * HD
    R = CH // 128
    BF = ml_dtypes.bfloat16
    x = np.asarray(x, np.float32).reshape(S, D)
    wq = np.asarray(wq, np.float32)
    wk = np.asarray(wk, np.float32)
    wv = np.asarray(wv, np.float32)
    wo = np.asarray(wo, np.float32)
    fc = np.asarray(freqs_cos, np.float32)
    fs = np.asarray(freqs_sin, np.float32)

    perm = _deinterleave_perm(HD)
    wq = wq * np.float32(SCALE)   # fold softmax scale into q projection
    xT = np.ascontiguousarray(x.T.astype(BF))           # [D, S] bf16
    cosP = np.ascontiguousarray(np.concatenate([fc.T, fc.T], 0))  # [128, S]
    sinP = np.ascontiguousarray(np.concatenate([-fs.T, fs.T], 0))
    # mask[t, r*CH + s] = 1 if 128*r + t <= s else 0
    tt = np.arange(128)[:, None]
    ss = np.arange(CH)[None, :]
    maskD = np.concatenate(
        [(128 * r + tt <= ss).astype(np.float32) for r in range(R)], axis=1)
    maskD = np.ascontiguousarray(maskD.astype(BF))      # [128, R*CH] bf16
    constD = np.concatenate(
        [np.eye(128, dtype=np.float32), np.ones((128, 128), np.float32)],
        axis=1).astype(BF)                              # [128, 256] bf16

    in_maps = []
    for c in range(n_cores):
        wq_c = wq[c * M:(c + 1) * M, :].reshape(hq, HD, D)[:, perm, :]
        wq_c = wq_c.reshape(M, D)
        wk_c = wk[c * HD:(c + 1) * HD, :][perm, :]
        wv_c = wv[c * HD:(c + 1) * HD, :]
        wqkvT = np.ascontiguousarray(
            np.concatenate([wq_c, wk_c, wv_c], axis=0).T.astype(BF))
        woT = np.ascontiguousarray(
            wo[:, c * M:(c + 1) * M].T.astype(BF))      # [M, D] bf16
        in_maps.append({
            "xT": xT, "wqkvT": wqkvT, "woT": woT, "constD": constD,
            "cosP": cosP, "sinP": sinP, "maskD": maskD,
        })
    return in_maps


_NC_CACHE = {}


def _get_module():
    if "nc" not in _NC_CACHE:
        _NC_CACHE["nc"] = build_module()
    return _NC_CACHE["nc"]


def run_on_cores(in_maps, trace=False):
    nc = _get_module()
    res = bass_utils.run_bass_kernel_spmd(
        nc, in_maps, core_ids=list(range(len(in_maps))), trace=trace)
    return res


def kernel(x, wq, wk, wv, wo, freqs_cos, freqs_sin):
    in_maps = host_prep(x, wq, wk, wv, wo, freqs_cos, freqs_sin)
    res = run_on_cores(in_maps)
    acc = None
    for r in res.results:
        o = r["outT"]
        acc = o.astype(np.float64) if acc is None else acc + o
    out = acc.T.astype(np.float32).reshape(1, SEQ, DIM)
    return out

